# revision 1
# baseline (speedup 1.0000x reference)
"""AOSPredictionLayer — Trainium2 Bass kernel (8 NeuronCores, expert-sharded).

Problem: pred[b,n] = <ui_mlp(concat(u,i))[b], relation_mlp[s[b,n]](concat(a,o)[b,n])>
with B=512, N=32, R=8 relations, MLP dims 512->512->256->128 (leaky_relu 0.01).

MoE routing: host groups the B*N = 16384 tokens by relation id; core e gets
exactly the tokens of relation e (per-relation weights sharded, UI MLP weights
replicated). On device (bf16 matmul inputs, fp32 PSUM accumulate):
  - shared UI MLP over all 512 batch rows (feature-major), transposed to
    token-major via 4 PE transposes into one PSUM bank + one wide DVE copy
  - the expert 3-layer MLP over its tokens in <=512-col chunks (feature-major,
    bias+leaky_relu fused on the Scalar engine)
  - one-hot gather of ui_emb[b(t)] per token (fp16 row-id compare on DVE
    tensor_scalar, 0/1 exact in bf16; gather by matmul)
  - dot product via elementwise multiply + ones-matmul partition reduce;
    pred stored straight from PSUM
All tensors reach SBUF via few large DMAs (one per tensor / token chunk,
[128, k, n] layouts) spread across the SP/Pool/Act queues; everything is bf16
on the wire except biases (f32) and row ids (fp16), halving the serialized
DMA-pipe time versus f32.
"""
import sys

for _p in ("/opt/trn_rl_repo", "/opt/pypackages"):
    if _p not in sys.path:
        sys.path.append(_p)

import numpy as np
import ml_dtypes

import concourse.tile as tile
from concourse import bacc, mybir
from concourse.masks import make_identity
from concourse.bass_utils import run_bass_kernel_spmd

F32 = mybir.dt.float32
BF16 = mybir.dt.bfloat16
F16 = mybir.dt.float16

BF16_NP = ml_dtypes.bfloat16

B = 512            # batch rows
N_TOK = 32         # tokens per batch row
IN1 = 256          # a/o embedding dim
IN2 = 256          # u/i embedding dim
HID = [512, 256, 128]
R = 8              # relations == cores
N_CORES = 8

D_AO = 2 * IN1
D_UI = 2 * IN2
DIMS = [D_AO] + HID          # [512, 512, 256, 128]
KC = [d // 128 for d in DIMS]            # k-chunks per layer input: [4,4,2,1]
MC = [d // 128 for d in DIMS[1:]]        # m-chunks per layer output: [4,2,1]
BOFF = [0, 4, 6]             # bias column offset per layer in the [128,7] pack


def _chunks(tp):
    """Split tp columns into <=512 chunks, near-equal, plus a small tail
    chunk (~128) so the final dot->store chain is short."""
    tail = 64 if tp > 640 else 0
    body = tp - tail
    nch = (body + 511) // 512
    base, rem = divmod(body, nch)
    sizes = [base + 1] * rem + [base] * (nch - rem)
    if tail:
        sizes.append(tail)
    out, s = [], 0
    for n in sizes:
        out.append((s, n))
        s += n
    return out


def _build_kernel(tp, chunk_cs):
    """Per-core Bacc graph for TP=tp padded tokens (identical on all cores;
    per-core data arrives via in_maps)."""
    nc = bacc.Bacc("TRN2", target_bir_lowering=False, debug=False,
                   num_devices=N_CORES)

    xT_d = nc.dram_tensor("xT", [D_AO, tp], BF16, kind="ExternalInput").ap()
    w_d = [nc.dram_tensor(f"w{i+1}", [DIMS[i], DIMS[i+1]], BF16,
                          kind="ExternalInput").ap() for i in range(3)]
    wu_d = [nc.dram_tensor(f"wu{i+1}", [DIMS[i], DIMS[i+1]], BF16,
                           kind="ExternalInput").ap() for i in range(3)]
    bp_d = nc.dram_tensor("bpack", [128, 7], F32, kind="ExternalInput").ap()
    bup_d = nc.dram_tensor("bupack", [128, 7], F32, kind="ExternalInput").ap()
    uiT_d = nc.dram_tensor("uiT", [D_UI, B], BF16, kind="ExternalInput").ap()
    bids_d = nc.dram_tensor("bids", [128, tp], F16, kind="ExternalInput").ap()
    iota_d = nc.dram_tensor("iota4", [128, 4], F32, kind="ExternalInput").ap()
    pred_d = nc.dram_tensor("pred", [1, tp], F32, kind="ExternalOutput").ap()

    BC = B // 128
    chunks = _chunks(tp)

    with tile.TileContext(nc) as tc:
        with (
            tc.tile_pool(name="wts", bufs=1) as wts,
            tc.tile_pool(name="ui", bufs=1) as uip,
            tc.tile_pool(name="xin", bufs=len(chunks)) as xin,
            tc.tile_pool(name="act", bufs=2) as actp,
            tc.tile_pool(name="mmps", bufs=8, space="PSUM") as mmps,
        ):
            # ---- PE warm-up: pe_busy_start is pinned at the first PE busy
            # moment and the clock reaches 2.4GHz 3us later, so keep the PE
            # grinding on throwaway matmuls while the first DMAs land ----
            ones = uip.tile([128, 1], BF16, tag="ones")
            nc.vector.memset(ones[:], 1.0)
            dum_in = uip.tile([128, 512], BF16, tag="dumin")
            nc.vector.memset(dum_in[:], 0.0)
            psdum = mmps.tile([128, 512], F32, tag="mm", name="psdum")
            for _ in range(7):
                nc.tensor.matmul(psdum[0:1, :], ones[:], dum_in[:],
                                 start=True, stop=True)

            # ---- DMAs: few and large, spread over the Act/SP/Pool queues ----
            # Act queue: UI weights (first UI matmul gates on wu1 halves),
            # then the LUT warm-up so the Lrelu table loads during the waits.
            wu1_h = []
            for h in range(2):
                t = wts.tile([128, 2, DIMS[1]], BF16, tag=f"wu1h{h}")
                nc.scalar.dma_start(
                    t[:], wu_d[0].rearrange("(k p) m -> p k m", p=128)[:, 2*h:2*h+2, :])
                wu1_h.append(t)
            lut_in = uip.tile([1, 1], F32, tag="lutin")
            nc.vector.memset(lut_in[:], 0.0)
            lut_out = uip.tile([1, 1], F32, tag="lutout")
            nc.scalar.activation(lut_out[:], lut_in[:],
                                 mybir.ActivationFunctionType.Lrelu,
                                 bias=0.0, scale=1.0, alpha=0.01)
            bu_s = wts.tile([128, 7], F32, tag="bu")
            nc.scalar.dma_start(bu_s[:], bup_d[:])

            # SP queue: UI input halves, then token chunks.
            uiT_h = []
            for h in range(2):
                t = wts.tile([128, 2, B], BF16, tag=f"uiTh{h}")
                nc.sync.dma_start(
                    t[:], uiT_d.rearrange("(k p) c -> p k c", p=128)[:, 2*h:2*h+2, :])
                uiT_h.append(t)
            wu23_s = []
            for i in (1, 2):
                t = wts.tile([128, KC[i], DIMS[i+1]], BF16, tag=f"wu{i+1}")
                nc.sync.dma_start(
                    t[:], wu_d[i].rearrange("(k p) m -> p k m", p=128))
                wu23_s.append(t)
            xcs = []
            xT_r = xT_d.rearrange("(k p) c -> p k c", p=128)
            for ci, (s0, n) in enumerate(chunks):
                t = xin.tile([128, 4, n], BF16, tag="x", name=f"x{ci}")
                nc.sync.dma_start(t[:], xT_r[:, :, s0:s0 + n])
                xcs.append(t)

            # Pool queue (SWDGE): expert weights, biases, row ids, identity.
            b_s = wts.tile([128, 7], F32, tag="b")
            nc.gpsimd.dma_start(b_s[:], bp_d[:])
            iota_s = wts.tile([128, 4], F32, tag="iota")
            nc.gpsimd.dma_start(iota_s[:], iota_d[:])
            w_s = []
            for i in range(3):
                t = wts.tile([128, KC[i], DIMS[i+1]], BF16, tag=f"w{i+1}")
                nc.gpsimd.dma_start(
                    t[:], w_d[i].rearrange("(k p) m -> p k m", p=128))
                w_s.append(t)
            bids_s = wts.tile([128, tp], F16, tag="bids")
            nc.gpsimd.dma_start(bids_s[:], bids_d[:])

            ident = uip.tile([128, 128], F32, tag="ident")
            make_identity(nc, ident[:])

            def mlp_layer(li, in_of_k, out_t, ws, bs, n_cols, order=None):
                """Feature-major layer li: out[:,m,:] = lrelu(sum_k w.T@in + b).
                in_of_k(k) -> [128, n_cols] AP; out_t [128, MC[li], n_cols].
                order: explicit (m, k) pass emission order (default m-major)."""
                kc, mc = KC[li], MC[li]
                if order is None:
                    order = [(m, k) for m in range(mc) for k in range(kc)]
                pss = [None] * mc
                seen = [0] * mc
                for m, k in order:
                    if pss[m] is None:
                        pss[m] = mmps.tile([128, 512], F32, tag="mm",
                                           name=f"mm_l{li}m{m}")
                    seen[m] += 1
                    nc.tensor.matmul(
                        pss[m][:, :n_cols], ws(k)[:, m * 128:(m + 1) * 128],
                        in_of_k(k), start=(seen[m] == 1), stop=(seen[m] == kc))
                    if seen[m] == kc:
                        nc.scalar.activation(
                            out_t[:, m, :], pss[m][:, :n_cols],
                            mybir.ActivationFunctionType.Lrelu,
                            bias=bs[:, BOFF[li] + m:BOFF[li] + m + 1],
                            scale=1.0, alpha=0.01)

            def chunk_mlp(ci, n, h1=None):
                if h1 is None:
                    h1 = actp.tile([128, 4, n], BF16, tag="h1",
                                   name=f"h1c{ci}")
                    mlp_layer(0, lambda k: xcs[ci][:, k, :], h1,
                              lambda k: w_s[0][:, k, :], b_s, n)
                h2 = actp.tile([128, 2, n], BF16, tag="h2", name=f"h2c{ci}")
                mlp_layer(1, lambda k: h1[:, k, :], h2,
                          lambda k: w_s[1][:, k, :], b_s, n,
                          order=[(m, k) for k in range(4) for m in range(2)])
                h3 = actp.tile([128, 1, n], BF16, tag="h3", name=f"h3c{ci}")
                mlp_layer(2, lambda k: h2[:, k, :], h3,
                          lambda k: w_s[2][:, k, :], b_s, n,
                          order=[(0, k) for k in range(2)])
                return h3

            # ---- UI MLP over all B rows (feature-major) ----
            # L1: the k0/k1 halves of uiT/wu1 land first; finish each m's
            # k0/k1 passes before the k2/k3 half arrives.
            ui1 = uip.tile([128, 4, B], BF16, tag="ui1")
            l1_order = [(m, k) for kk in (0, 2) for m in range(4)
                        for k in (kk, kk + 1)]
            mlp_layer(0, lambda k: uiT_h[k // 2][:, k % 2, :], ui1,
                      lambda k: wu1_h[k // 2][:, k % 2, :], bu_s, B,
                      order=l1_order)
            # L2/L3 k-major: k inputs are the L1/L2 acts, which retire in
            # m order on the Act queue — k-major consumes them just-in-time.
            ui2 = uip.tile([128, 2, B], BF16, tag="ui2")
            mlp_layer(1, lambda k: ui1[:, k, :], ui2,
                      lambda k: wu23_s[0][:, k, :], bu_s, B,
                      order=[(m, k) for k in range(4) for m in range(2)])
            ui3 = uip.tile([128, 1, B], F32, tag="ui3")
            mlp_layer(2, lambda k: ui2[:, k, :], ui3,
                      lambda k: wu23_s[1][:, k, :], bu_s, B,
                      order=[(0, k) for k in range(2)])

            # chunk 0 expert MLP layer 1 keeps the PE fed while the ui3
            # act/transpose chain completes
            h1_c0 = actp.tile([128, 4, chunks[0][1]], BF16, tag="h1",
                              name="h1c0")
            mlp_layer(0, lambda k: xcs[0][:, k, :], h1_c0,
                      lambda k: w_s[0][:, k, :], b_s, chunks[0][1])

            # transpose ui3 [128d x B] -> token-major [128b x BC x 128d]:
            # 4 PE transposes into one PSUM bank, one wide DVE copy out.
            tps = mmps.tile([128, 512], F32, tag="mm", name="tps")
            for c in range(BC):
                nc.tensor.transpose(tps[:, c * 128:(c + 1) * 128],
                                    ui3[:, 0, c * 128:(c + 1) * 128], ident[:])
            ui3_tok = uip.tile([128, BC, 128], BF16, tag="ui3tok")
            nc.vector.tensor_copy(ui3_tok[:], tps[:])

            # ---- token chunks, finishes software-pipelined one behind ----
            def chunk_finish(ci, h3):
                s0, n = chunks[ci]
                cs = chunk_cs[ci]
                # one-hot[b, t] = (b == bids[t]) for the b-chunks present
                oh = actp.tile([128, BC, n], BF16, tag="oh", name=f"oh{ci}")
                for c in cs:
                    nc.vector.tensor_scalar(
                        out=oh[:, c, :], in0=bids_s[:, s0:s0 + n],
                        scalar1=iota_s[:, c:c + 1], scalar2=None,
                        op0=mybir.AluOpType.is_equal)
                # gathered ui columns: uig = ui3_tok.T @ oh  (exact selection)
                psg = mmps.tile([128, 512], F32, tag="mm")
                for j, c in enumerate(cs):
                    nc.tensor.matmul(psg[:, :n], ui3_tok[:, c, :], oh[:, c, :],
                                     start=(j == 0), stop=(j == len(cs) - 1))
                # pred = ones.T @ (h3 * uig)   (partition reduce over d=128)
                prod = actp.tile([128, n], BF16, tag="prod", name=f"prod{ci}")
                nc.vector.tensor_tensor(out=prod[:], in0=h3[:, 0, :],
                                        in1=psg[:, :n],
                                        op=mybir.AluOpType.mult)
                psd = mmps.tile([128, 512], F32, tag="mm", name=f"psd{ci}")
                nc.tensor.matmul(psd[0:1, :n], ones[:], prod[:],
                                 start=True, stop=True)
                pc = actp.tile([1, n], F32, tag="predc", name=f"pc{ci}")
                nc.vector.tensor_copy(pc[:], psd[0:1, :n])
                nc.sync.dma_start(pred_d[:, s0:s0 + n], pc[:])

            for ci, (s0, n) in enumerate(chunks):
                chunk_finish(ci, chunk_mlp(ci, n,
                                           h1=h1_c0 if ci == 0 else None))

    nc.compile()
    return nc


def _prepare(u_emb, i_emb, a_emb, o_emb, s):
    """Host-side sharding: route tokens to cores by relation id."""
    s_flat = np.asarray(s).reshape(-1).astype(np.int64)
    n_tokens = s_flat.shape[0]
    X = np.concatenate(
        [np.asarray(a_emb, dtype=np.float32).reshape(n_tokens, IN1),
         np.asarray(o_emb, dtype=np.float32).reshape(n_tokens, IN1)],
        axis=1).astype(BF16_NP)
    uiT = np.ascontiguousarray(
        np.concatenate([np.asarray(u_emb, dtype=np.float32),
                        np.asarray(i_emb, dtype=np.float32)],
                       axis=1).astype(BF16_NP).T)

    idx = [np.flatnonzero(s_flat == e) for e in range(R)]
    tp = max(256, -(-max(max(len(ix) for ix in idx), 1) // 4) * 4)

    iota4 = np.ascontiguousarray(
        (np.arange(128, dtype=np.float32)[:, None] +
         128.0 * np.arange(4, dtype=np.float32)[None, :]))

    in_maps = []
    chunks = _chunks(tp)
    chunk_cs = [set() for _ in chunks]
    for e in range(R):
        # sort tokens by batch row within the expert: narrows per-chunk b-range
        order = np.argsort(idx[e] // N_TOK, kind="stable")
        idx[e] = idx[e][order]
        ix = idx[e]
        pad = np.full(tp, n_tokens - 1, dtype=np.int64)
        pad[:len(ix)] = ix
        xT = np.ascontiguousarray(X[pad].T)
        b_of_tok = pad // N_TOK
        bids = np.ascontiguousarray(np.broadcast_to(
            b_of_tok.astype(np.float16)[None, :], (128, tp)))
        cg = b_of_tok // 128
        for ci, (s0, n) in enumerate(chunks):
            for c in np.unique(cg[s0:s0 + n]):
                chunk_cs[ci].add(int(c))
        in_maps.append({"xT": xT, "uiT": uiT, "bids": bids, "iota4": iota4})
    chunk_cs = [sorted(cset) for cset in chunk_cs]
    return in_maps, idx, tp, chunk_cs


def _pack_bias(bs):
    """Stack per-layer bias vectors into a [128, n_cols] f32 pack."""
    cols = []
    for b in bs:
        cols.append(np.asarray(b, dtype=np.float32).reshape(-1, 128).T)
    return np.ascontiguousarray(np.concatenate(cols, axis=1))


def kernel(u_emb, i_emb, a_emb, o_emb, s,
           W1, b1, W2, b2, W3, b3,
           Wu1, bu1, Wu2, bu2, Wu3, bu3):
    in_maps, idx, tp, chunk_cs = _prepare(u_emb, i_emb, a_emb, o_emb, s)
    ws = {"W1": W1, "W2": W2, "W3": W3, "Wu1": Wu1, "Wu2": Wu2, "Wu3": Wu3}
    bupack = _pack_bias([bu1, bu2, bu3])
    for e in range(R):
        m = in_maps[e]
        for li in range(3):
            m[f"w{li+1}"] = np.ascontiguousarray(
                np.asarray(ws[f"W{li+1}"][e], dtype=np.float32).astype(BF16_NP))
            m[f"wu{li+1}"] = np.ascontiguousarray(
                np.asarray(ws[f"Wu{li+1}"], dtype=np.float32).astype(BF16_NP))
        m["bpack"] = _pack_bias([b1[e], b2[e], b3[e]])
        m["bupack"] = bupack

    nc = _build_kernel(tp, chunk_cs)
    res = run_bass_kernel_spmd(nc, in_maps, core_ids=list(range(N_CORES)))

    s_arr = np.asarray(s)
    out = np.zeros(s_arr.size, dtype=np.float32)
    for e in range(R):
        pred = res.results[e]["pred"].reshape(-1)
        out[idx[e]] = pred[:len(idx[e])]
    return out.reshape(s_arr.shape)



# revision 7
# speedup vs baseline: 1.4451x; 1.4451x over previous
"""AOSPredictionLayer — Trainium2 Bass kernel (8 NeuronCores, expert-sharded).

pred[b,n] = <ui_mlp(concat(u,i))[b], relation_mlp[s[b,n]](concat(a,o)[b,n])>
B=512, N=32, R=8, MLP dims 512->512->256->128 (leaky_relu 0.01).

Core e gets the tokens of relation e (sorted by batch row, padded to tp).
Device dataflow per core:
  - L1 (expert + UI) in fp8e4m3 DoubleRow, 3-term error-compensated:
    w' = 16*W split hi/lo, x split hi/lo (host-side); wh*xh + wl*xh + wh*xl
    accumulated in fp32 PSUM; activation applies scale=1/16.
  - expert L2/L3 in fp8 DoubleRow 2-term (wh+wl)*h with h1/h2 written as
    fp8 by the activations (UI L2/L3 stay bf16 for accuracy headroom).
  - Gather/dot finish per 512-token chunk: one-hot (b == iota+128c) rows,
    psg = ui3_tok^T @ oh (accumulated across the chunk's b-groups in one
    PSUM group so overlapping column spans add), prod = h3 * psg on DVE,
    pred = ones^T @ prod, copy on Pool, DMA on SP.
  - Software pipeline: period k emits L1(ck) | L2(ck-1)+L3(ck-1) |
    psg/psd(ck-2) interleaved so the in-order PE stream never waits.
    Activations split: h1m0-2, h2, h3 on Act engine; h1m3 on DVE.
"""
import sys

for _p in ("/opt/trn_rl_repo", "/opt/pypackages"):
    if _p not in sys.path:
        sys.path.append(_p)

import numpy as np
import ml_dtypes

import concourse.tile as tile
from concourse import bacc, mybir
from concourse.masks import make_identity
from concourse.bass_utils import run_bass_kernel_spmd

F32 = mybir.dt.float32
BF16 = mybir.dt.bfloat16
F16 = mybir.dt.float16
E4 = mybir.dt.float8e4

BF16_NP = ml_dtypes.bfloat16
E4_NP = ml_dtypes.float8_e4m3

B = 512
N_TOK = 32
IN1 = 256
HID = [512, 256, 128]
R = 8
N_CORES = 8

D_AO = 2 * IN1                     # 512
DIMS = [D_AO] + HID                # [512, 512, 256, 128]
WSCALE = 16.0
DR = mybir.MatmulPerfMode.DoubleRow
LR = 0.01                          # leaky-relu slope
BOFF = [0, 4, 6]                   # bias col offset per layer in [128,7]


def _chunks(tp):
    out, s = [], 0
    while tp - s > 512:
        out.append((s, 512))
        s += 512
    out.append((s, tp - s))
    return out


def _halves(n):
    if n <= 256:
        return [(0, n)]
    return [(0, 256), (256, n)]


def _build_kernel(tp, pieces):
    """pieces[ci] = list of (group, lo, hi): union spans of b//128 groups
    in chunk ci across all cores (one-hot zeroes wrong-group terms)."""
    nc = bacc.Bacc("TRN2", target_bir_lowering=False, debug=False,
                   num_devices=N_CORES)

    xh_d = nc.dram_tensor("xh", [128, 4, tp], E4, kind="ExternalInput").ap()
    xl_d = nc.dram_tensor("xl", [128, 4, tp], E4, kind="ExternalInput").ap()
    wd = {}
    for nm, shp in (("wh1", [128, 4, 512]), ("wl1", [128, 4, 512]),
                    ("wh2", [128, 4, 256]), ("wl2", [128, 4, 256]),
                    ("wh3", [128, 2, 128]), ("wl3", [128, 2, 128]),
                    ("uiTh", [128, 4, B]), ("uiTl", [128, 4, B]),
                    ("wuh1", [128, 4, 512]), ("wul1", [128, 4, 512])):
        wd[nm] = nc.dram_tensor(nm, shp, E4, kind="ExternalInput").ap()
    wu2_d = nc.dram_tensor("wu2", [128, 4, 256], BF16,
                           kind="ExternalInput").ap()
    wu3_d = nc.dram_tensor("wu3", [128, 2, 128], BF16,
                           kind="ExternalInput").ap()
    bp_d = nc.dram_tensor("bpack", [128, 7], F32, kind="ExternalInput").ap()
    bup_d = nc.dram_tensor("bupack", [128, 7], F32, kind="ExternalInput").ap()
    bids_d = nc.dram_tensor("bids", [128, tp], F16, kind="ExternalInput").ap()
    iota_d = nc.dram_tensor("iota4", [128, 4], F32, kind="ExternalInput").ap()
    pred_d = nc.dram_tensor("pred", [1, tp], F32, kind="ExternalOutput").ap()

    chunks = _chunks(tp)
    NCH = len(chunks)

    with tile.TileContext(nc) as tc:
        with (
            tc.tile_pool(name="wts", bufs=1) as wts,
            tc.tile_pool(name="xin", bufs=1) as xin,
            tc.tile_pool(name="uip", bufs=1) as uip,
            tc.tile_pool(name="actp", bufs=2) as actp,
            tc.tile_pool(name="finp", bufs=2) as finp,
            tc.tile_pool(name="mmps", bufs=8, space="PSUM") as mmps,
        ):
            # ---- tiny PE warm-up (pins pe_busy_start; ~free) ----
            dum = uip.tile([1, 8], BF16, tag="dum")
            nc.vector.memset(dum[:], 0.0)
            psw = mmps.tile([128, 512], F32, tag="mm", name="psw")
            for _ in range(2):
                nc.tensor.matmul(psw[0:1, 0:8], dum[0:1, 0:1], dum[:],
                                 start=True, stop=True)

            # ---- DMAs ----
            # SP queue: UI moving halves, UI weights, x-lo, bids.
            t_uiTh = wts.tile([128, 4, B], E4, tag="uiTh")
            nc.sync.dma_start(t_uiTh[:], wd["uiTh"])
            t_uiTl = wts.tile([128, 4, B], E4, tag="uiTl")
            nc.sync.dma_start(t_uiTl[:], wd["uiTl"])
            t_wu2 = wts.tile([128, 4, 256], BF16, tag="wu2")
            nc.sync.dma_start(t_wu2[:], wu2_d)
            t_wu3 = wts.tile([128, 2, 128], BF16, tag="wu3")
            nc.sync.dma_start(t_wu3[:], wu3_d)
            t_xl = xin.tile([128, 4, tp], E4, tag="xl")
            nc.sync.dma_start(t_xl[:], xl_d)
            t_bids = wts.tile([128, tp], F16, tag="bids")
            nc.sync.dma_start(t_bids[:], bids_d)

            # Act queue: small UI weights, then LUT warm-up.
            t_wul1 = wts.tile([128, 4, 512], E4, tag="wul1")
            nc.scalar.dma_start(t_wul1[:], wd["wul1"])
            t_bu = wts.tile([128, 7], F32, tag="bu")
            nc.scalar.dma_start(t_bu[:], bup_d)
            lut = uip.tile([1, 1], F32, tag="lut")
            nc.vector.memset(lut[:], 0.0)
            nc.scalar.activation(lut[:], lut[:],
                                 mybir.ActivationFunctionType.Lrelu,
                                 bias=0.0, scale=1.0, alpha=LR)

            # Pool queue: expert weights, x-hi, biases, iota.
            t_wuh1 = wts.tile([128, 4, 512], E4, tag="wuh1")
            nc.gpsimd.dma_start(t_wuh1[:], wd["wuh1"])
            t_wh1 = wts.tile([128, 4, 512], E4, tag="wh1")
            nc.gpsimd.dma_start(t_wh1[:], wd["wh1"])
            t_xh = xin.tile([128, 4, tp], E4, tag="xh")
            nc.gpsimd.dma_start(t_xh[:], xh_d)
            t_wl1 = wts.tile([128, 4, 512], E4, tag="wl1")
            nc.gpsimd.dma_start(t_wl1[:], wd["wl1"])
            t_b = wts.tile([128, 7], F32, tag="b")
            nc.gpsimd.dma_start(t_b[:], bp_d)
            t_iota = wts.tile([128, 4], F32, tag="iota")
            nc.gpsimd.dma_start(t_iota[:], iota_d)
            t_wh2 = wts.tile([128, 4, 256], E4, tag="wh2")
            nc.gpsimd.dma_start(t_wh2[:], wd["wh2"])
            t_wl2 = wts.tile([128, 4, 256], E4, tag="wl2")
            nc.gpsimd.dma_start(t_wl2[:], wd["wl2"])
            t_wh3 = wts.tile([128, 2, 128], E4, tag="wh3")
            nc.gpsimd.dma_start(t_wh3[:], wd["wh3"])
            t_wl3 = wts.tile([128, 2, 128], E4, tag="wl3")
            nc.gpsimd.dma_start(t_wl3[:], wd["wl3"])

            ones = uip.tile([128, 1], BF16, tag="ones")
            nc.vector.memset(ones[:], 1.0)
            ident = uip.tile([128, 128], F32, tag="ident")
            make_identity(nc, ident[:])

            # ---- emission helpers ----
            def mm3(ps, whT, wlT, mvh, mvl, pair, m, lo, hi):
                """3-term fp8 DoubleRow group piece for one (m, colhalf)."""
                wsl_h = whT[:, 2 * pair:2 * pair + 2, m * 128:(m + 1) * 128]
                wsl_l = wlT[:, 2 * pair:2 * pair + 2, m * 128:(m + 1) * 128]
                xs_h = mvh[:, 2 * pair:2 * pair + 2, lo:hi]
                xs_l = mvl[:, 2 * pair:2 * pair + 2, lo:hi]
                o = ps[:, lo:hi]
                nc.tensor.matmul(o, wsl_h, xs_h, start=(pair == 0),
                                 stop=False, perf_mode=DR)
                nc.tensor.matmul(o, wsl_l, xs_h, start=False, stop=False,
                                 perf_mode=DR)
                nc.tensor.matmul(o, wsl_h, xs_l, start=False,
                                 stop=(pair == 1), perf_mode=DR)

            def mm2(ps, whT, wlT, mv, pairs, m, lo, hi):
                """2-term fp8 DoubleRow group: (wh+wl)*h."""
                o = ps[:, lo:hi]
                for p in range(pairs):
                    wsl_h = whT[:, 2 * p:2 * p + 2, m * 128:(m + 1) * 128]
                    wsl_l = wlT[:, 2 * p:2 * p + 2, m * 128:(m + 1) * 128]
                    xs = mv[:, 2 * p:2 * p + 2, lo:hi]
                    nc.tensor.matmul(o, wsl_h, xs, start=(p == 0),
                                     stop=False, perf_mode=DR)
                    nc.tensor.matmul(o, wsl_l, xs, start=False,
                                     stop=(p == pairs - 1), perf_mode=DR)

            import itertools
            _ctr = itertools.count()

            def act(out, ps, bias_t, bcol, n, scale, dve=False):
                """bias + leaky-relu (+descale); on Act engine or DVE."""
                if not dve:
                    nc.scalar.activation(
                        out, ps, mybir.ActivationFunctionType.Lrelu,
                        bias=bias_t[:, bcol:bcol + 1], scale=scale, alpha=LR)
                else:
                    tmp = actp.tile([128, 512], F32, tag="dvetmp",
                                    name=f"dt{next(_ctr)}")
                    nc.vector.tensor_scalar(
                        out=tmp[:, :n], in0=ps, scalar1=scale,
                        scalar2=bias_t[:, bcol:bcol + 1],
                        op0=mybir.AluOpType.mult, op1=mybir.AluOpType.add)
                    nc.vector.scalar_tensor_tensor(
                        out=out, in0=tmp[:, :n], scalar=LR, in1=tmp[:, :n],
                        op0=mybir.AluOpType.mult, op1=mybir.AluOpType.max)

            S16 = 1.0 / WSCALE

            # ---------------- UI phase ----------------
            ps_ui1 = [mmps.tile([128, 512], F32, tag="mm", name=f"psu1m{m}")
                      for m in range(4)]
            ui1 = uip.tile([128, 4, B], BF16, tag="ui1")
            for m in range(4):
                for lo, hi in _halves(B):
                    mm3(ps_ui1[m], t_wuh1, t_wul1, t_uiTh, t_uiTl, 0, m, lo, hi)
                    mm3(ps_ui1[m], t_wuh1, t_wul1, t_uiTh, t_uiTl, 1, m, lo, hi)
                act(ui1[:, m, :], ps_ui1[m][:, :B], t_bu, BOFF[0] + m, B, S16,
                    dve=(m == 3))

            ps_ui2 = [mmps.tile([128, 512], F32, tag="mm", name=f"psu2m{m}")
                      for m in range(2)]
            ui2 = uip.tile([128, 2, B], BF16, tag="ui2")
            for k in range(4):
                for m in range(2):
                    nc.tensor.matmul(
                        ps_ui2[m][:, :B], t_wu2[:, k, m * 128:(m + 1) * 128],
                        ui1[:, k, :], start=(k == 0), stop=(k == 3))
            for m in range(2):
                act(ui2[:, m, :], ps_ui2[m][:, :B], t_bu, BOFF[1] + m, B, 1.0)

            ps_ui3 = mmps.tile([128, 512], F32, tag="mm", name="psu3")
            ui3 = uip.tile([128, 1, B], F32, tag="ui3")
            for k in range(2):
                nc.tensor.matmul(ps_ui3[:, :B], t_wu3[:, k, :], ui2[:, k, :],
                                 start=(k == 0), stop=(k == 1))
            act(ui3[:, 0, :], ps_ui3[:, :B], t_bu, BOFF[2], B, 1.0)

            # transpose ui3 -> token-major [128b, 4, 128d]
            tps = mmps.tile([128, 512], F32, tag="mm", name="tps")
            for c in range(4):
                nc.tensor.transpose(tps[:, c * 128:(c + 1) * 128],
                                    ui3[:, 0, c * 128:(c + 1) * 128], ident[:])
            ui3_tok = uip.tile([128, 4, 128], BF16, tag="ui3tok")
            nc.vector.tensor_copy(ui3_tok[:], tps[:])

            # ---------------- expert chunk pipeline ----------------
            ps_l1 = [[None] * 4 for _ in range(NCH)]
            ps_l2 = [[None] * 2 for _ in range(NCH)]
            ps_l3 = [None] * NCH
            ps_dot = [None] * NCH
            h1 = [None] * NCH
            h2 = [None] * NCH
            h3 = [None] * NCH
            ohs = [None] * NCH
            prods = [None] * NCH
            pcs = [None] * NCH

            def alloc_chunk(ci):
                for m in range(4):
                    ps_l1[ci][m] = mmps.tile([128, 512], F32, tag="mm",
                                             name=f"ps1c{ci}m{m}")
                for m in range(2):
                    ps_l2[ci][m] = mmps.tile([128, 512], F32, tag="mm",
                                             name=f"ps2c{ci}m{m}")
                ps_l3[ci] = mmps.tile([128, 512], F32, tag="mm",
                                      name=f"ps3c{ci}")
                ps_dot[ci] = mmps.tile([128, 512], F32, tag="mm",
                                       name=f"psdc{ci}")
                h1[ci] = actp.tile([128, 4, 512], E4, tag="h1",
                                   name=f"h1c{ci}")
                h2[ci] = actp.tile([128, 2, 512], E4, tag="h2",
                                   name=f"h2c{ci}")
                h3[ci] = actp.tile([128, 1, 512], BF16, tag="h3",
                                   name=f"h3c{ci}")
                ohs[ci] = finp.tile([128, 4, 512], BF16, tag="oh",
                                    name=f"oh{ci}")
                prods[ci] = finp.tile([128, 512], BF16, tag="prod",
                                      name=f"prod{ci}")
                pcs[ci] = finp.tile([1, 512], F32, tag="pc", name=f"pc{ci}")

            def e_l1_group(ci, m):
                s0, n = chunks[ci]
                for lo, hi in _halves(n):
                    mm3(ps_l1[ci][m], t_wh1, t_wl1,
                        t_xh[:, :, s0:s0 + n], t_xl[:, :, s0:s0 + n],
                        0, m, lo, hi)
                    mm3(ps_l1[ci][m], t_wh1, t_wl1,
                        t_xh[:, :, s0:s0 + n], t_xl[:, :, s0:s0 + n],
                        1, m, lo, hi)

            def e_l1_act(ci, m, dve=False):
                n = chunks[ci][1]
                act(h1[ci][:, m, :n], ps_l1[ci][m][:, :n], t_b,
                    BOFF[0] + m, n, S16, dve=dve)

            def e_l2_pair(ci, m):
                n = chunks[ci][1]
                for lo, hi in _halves(n):
                    mm2(ps_l2[ci][m], t_wh2, t_wl2, h1[ci][:, :, :n],
                        2, m, lo, hi)

            def e_l2_act(ci, m):
                n = chunks[ci][1]
                act(h2[ci][:, m, :n], ps_l2[ci][m][:, :n], t_b,
                    BOFF[1] + m, n, S16)

            def e_l3(ci):
                n = chunks[ci][1]
                for lo, hi in _halves(n):
                    mm2(ps_l3[ci], t_wh3, t_wl3, h2[ci][:, :, :n], 1, 0,
                        lo, hi)

            def e_h3_act(ci):
                n = chunks[ci][1]
                act(h3[ci][:, 0, :n], ps_l3[ci][:, :n], t_b, BOFF[2], n, S16,
                    dve=True)

            def e_oh(ci):
                s0, n = chunks[ci]
                for j, (g, lo, hi) in enumerate(pieces[ci]["groups"]):
                    nc.vector.tensor_scalar(
                        out=ohs[ci][:, j, lo:hi],
                        in0=t_bids[:, s0 + lo:s0 + hi],
                        scalar1=t_iota[:, g:g + 1], scalar2=None,
                        op0=mybir.AluOpType.is_equal)

            def e_psg(ci):
                mms = pieces[ci]["mms"]
                for j, (slot, lo, hi) in enumerate(mms):
                    g = pieces[ci]["groups"][slot][0]
                    nc.tensor.matmul(
                        ps_dot[ci][:, lo:hi], ui3_tok[:, g, :],
                        ohs[ci][:, slot, lo:hi],
                        start=(j == 0), stop=(j == len(mms) - 1))

            def e_prod(ci):
                n = chunks[ci][1]
                nc.vector.tensor_tensor(
                    out=prods[ci][:, :n], in0=h3[ci][:, 0, :n],
                    in1=ps_dot[ci][:, :n], op=mybir.AluOpType.mult)

            def e_psd(ci):
                n = chunks[ci][1]
                nc.tensor.matmul(ps_dot[ci][0:1, :n], ones[:],
                                 prods[ci][:, :n], start=True, stop=True)

            def e_pc_dma(ci):
                # Pool/GpSimd cannot read PSUM on TRN2 — copy on DVE.
                s0, n = chunks[ci]
                nc.vector.tensor_copy(pcs[ci][:, :n], ps_dot[ci][0:1, :n])
                nc.sync.dma_start(pred_d[:, s0:s0 + n], pcs[ci][:, :n])

            for ci in range(NCH):
                alloc_chunk(ci)
                e_oh(ci)

            # Pipeline periods: PE stream per k:
            #  L1(k)m0 | L2(k-1)m0 | psg(k-2) | L1(k)m1 | L2(k-1)m1 |
            #  L1(k)m2 | psd(k-2) | L1(k)m3 | L3(k-1)
            for k in range(NCH + 2):
                ck, cp, cq = k, k - 1, k - 2
                if ck < NCH:
                    e_l1_group(ck, 0)
                if cp >= 0 and cp < NCH:
                    e_l2_pair(cp, 0)
                    e_l2_act(cp, 0)
                if cq >= 0 and cq < NCH:
                    e_psg(cq)
                if ck < NCH:
                    e_l1_group(ck, 1)
                    e_l1_act(ck, 0)
                if cp >= 0 and cp < NCH:
                    e_l2_pair(cp, 1)
                    e_l2_act(cp, 1)
                if cq >= 0 and cq < NCH:
                    e_prod(cq)
                if ck < NCH:
                    e_l1_group(ck, 2)
                    e_l1_act(ck, 1)
                if cq >= 0 and cq < NCH:
                    e_psd(cq)
                    e_pc_dma(cq)
                if ck < NCH:
                    e_l1_group(ck, 3)
                    e_l1_act(ck, 2)
                    e_l1_act(ck, 3)
                if cp >= 0 and cp < NCH:
                    e_l3(cp)
                    e_h3_act(cp)

    nc.compile()
    return nc


def _prepare(u_emb, i_emb, a_emb, o_emb, s):
    """Host-side sharding + fp8 hi/lo splits + layouts."""
    s_flat = np.asarray(s).reshape(-1).astype(np.int64)
    n_tokens = s_flat.shape[0]
    X = np.concatenate(
        [np.asarray(a_emb, dtype=np.float32).reshape(n_tokens, IN1),
         np.asarray(o_emb, dtype=np.float32).reshape(n_tokens, IN1)],
        axis=1)
    uiT = np.ascontiguousarray(
        np.concatenate([np.asarray(u_emb, dtype=np.float32),
                        np.asarray(i_emb, dtype=np.float32)], axis=1).T)

    idx = [np.flatnonzero(s_flat == e) for e in range(R)]
    tp = max(520, -(-max(max(len(ix) for ix in idx), 1) // 8) * 8)
    chunks = _chunks(tp)

    iota4 = np.ascontiguousarray(
        (np.arange(128, dtype=np.float32)[:, None]
         + 128.0 * np.arange(4, dtype=np.float32)[None, :]))

    def lay(a, kc):
        return np.ascontiguousarray(a.reshape(kc, 128, -1).transpose(1, 0, 2))

    uiTh = uiT.astype(E4_NP)
    uiTl = (uiT - uiTh.astype(np.float32)).astype(E4_NP)

    in_maps = []
    runs_all = [[] for _ in chunks]
    for e in range(R):
        order = np.argsort(idx[e] // N_TOK, kind="stable")
        idx[e] = idx[e][order]
        ix = idx[e]
        pad = np.full(tp, n_tokens - 1, dtype=np.int64)
        pad[:len(ix)] = ix
        xT = np.ascontiguousarray(X[pad].T)          # [512, tp]
        xh = xT.astype(E4_NP)
        xl = (xT - xh.astype(np.float32)).astype(E4_NP)
        b_of_tok = pad // N_TOK
        gid = b_of_tok // 128
        bids = np.ascontiguousarray(np.broadcast_to(
            b_of_tok.astype(np.float16)[None, :], (128, tp)))
        for ci, (s0, n) in enumerate(chunks):
            g = gid[s0:s0 + n]
            start = 0
            for j in range(1, n + 1):
                if j == n or g[j] != g[start]:
                    runs_all[ci].append((int(g[start]), start, j))
                    start = j
        in_maps.append({"xh": lay(xh, 4), "xl": lay(xl, 4), "bids": bids,
                        "iota4": iota4, "uiTh": lay(uiTh, 4),
                        "uiTl": lay(uiTl, 4)})

    # union spans per (chunk, group) across cores; psg matmuls split at
    # overlap boundaries (each mm range uniformly fresh or accumulating)
    pieces = []
    for ci in range(len(chunks)):
        spans = {}
        for g, lo, hi in runs_all[ci]:
            if g in spans:
                spans[g] = (min(spans[g][0], lo), max(spans[g][1], hi))
            else:
                spans[g] = (lo, hi)
        groups = sorted((g, lo, hi) for g, (lo, hi) in spans.items())
        bounds = sorted({b for _, lo, hi in groups for b in (lo, hi)})
        mms = []
        for a, b in zip(bounds[:-1], bounds[1:]):
            for slot, (g, lo, hi) in enumerate(groups):
                if lo <= a and b <= hi:
                    mms.append((slot, a, b))
        pieces.append({"groups": groups, "mms": mms})

    return in_maps, idx, tp, pieces


def _split16(w):
    ws = np.asarray(w, dtype=np.float32) * WSCALE
    wh = ws.astype(E4_NP)
    wl = (ws - wh.astype(np.float32)).astype(E4_NP)
    return wh, wl


def _lay(a, kc):
    return np.ascontiguousarray(
        np.asarray(a).reshape(kc, 128, -1).transpose(1, 0, 2))


def _pack_bias(bs):
    cols = []
    for b in bs:
        cols.append(np.asarray(b, dtype=np.float32).reshape(-1, 128).T)
    return np.ascontiguousarray(np.concatenate(cols, axis=1))


def _add_weights(m, e, W1, b1, W2, b2, W3, b3, Wu1, bu1, Wu2, bu2, Wu3, bu3,
                 bupack, wu_cache):
    for nm, w, kc in (("w1", W1[e], 4), ("w2", W2[e], 4), ("w3", W3[e], 2)):
        wh, wl = _split16(w)
        m[nm.replace("w", "wh")] = _lay(wh, kc)
        m[nm.replace("w", "wl")] = _lay(wl, kc)
    if not wu_cache:
        wh, wl = _split16(Wu1)
        wu_cache["wuh1"] = _lay(wh, 4)
        wu_cache["wul1"] = _lay(wl, 4)
        wu_cache["wu2"] = _lay(
            np.asarray(Wu2, dtype=np.float32).astype(BF16_NP), 4)
        wu_cache["wu3"] = _lay(
            np.asarray(Wu3, dtype=np.float32).astype(BF16_NP), 2)
    m.update(wu_cache)
    m["bpack"] = _pack_bias([b1[e], b2[e], b3[e]])
    m["bupack"] = bupack


def kernel(u_emb, i_emb, a_emb, o_emb, s,
           W1, b1, W2, b2, W3, b3,
           Wu1, bu1, Wu2, bu2, Wu3, bu3):
    in_maps, idx, tp, pieces = _prepare(u_emb, i_emb, a_emb, o_emb, s)
    bupack = _pack_bias([bu1, bu2, bu3])
    wu_cache = {}
    for e in range(R):
        _add_weights(in_maps[e], e, W1, b1, W2, b2, W3, b3,
                     Wu1, bu1, Wu2, bu2, Wu3, bu3, bupack, wu_cache)

    nc = _build_kernel(tp, pieces)
    res = run_bass_kernel_spmd(nc, in_maps, core_ids=list(range(N_CORES)))

    s_arr = np.asarray(s)
    out = np.zeros(s_arr.size, dtype=np.float32)
    for e in range(R):
        pred = res.results[e]["pred"].reshape(-1)
        out[idx[e]] = pred[:len(idx[e])]
    return out.reshape(s_arr.shape)


# revision 13
# speedup vs baseline: 1.5047x; 1.0412x over previous
"""AOSPredictionLayer — Trainium2 Bass kernel (8 NeuronCores, expert-sharded).

pred[b,n] = <ui_mlp(concat(u,i))[b], relation_mlp[s[b,n]](concat(a,o)[b,n])>
B=512, N=32, R=8, MLP dims 512->512->256->128 (leaky_relu 0.01).

Core e gets the tokens of relation e (sorted by batch row, padded to tp).
Device dataflow per core:
  - L1 (expert + UI) in fp8e4m3 DoubleRow, 3-term error-compensated:
    w' = 16*W split hi/lo, x split hi/lo (host-side); wh*xh + wl*xh + wh*xl
    accumulated in fp32 PSUM; activation applies scale=1/16.
  - expert L2/L3 in fp8 DoubleRow 2-term (wh+wl)*h with h1/h2 written as
    fp8 by the activations (UI L2/L3 stay bf16 for accuracy headroom).
  - Gather/dot finish per 512-token chunk: one-hot (b == iota+128c) rows,
    psg = ui3_tok^T @ oh (accumulated across the chunk's b-groups in one
    PSUM group so overlapping column spans add), prod = h3 * psg on DVE,
    pred = ones^T @ prod, copy on Pool, DMA on SP.
  - Software pipeline: period k emits L1(ck) | L2(ck-1)+L3(ck-1) |
    psg/psd(ck-2) interleaved so the in-order PE stream never waits.
    Activations split: h1m0-2, h2, h3 on Act engine; h1m3 on DVE.
"""
import sys

for _p in ("/opt/trn_rl_repo", "/opt/pypackages"):
    if _p not in sys.path:
        sys.path.append(_p)

import numpy as np
import ml_dtypes

import concourse.tile as tile
from concourse import bacc, mybir
from concourse.masks import make_identity
from concourse.bass_utils import run_bass_kernel_spmd

F32 = mybir.dt.float32
BF16 = mybir.dt.bfloat16
F16 = mybir.dt.float16
E4 = mybir.dt.float8e4

BF16_NP = ml_dtypes.bfloat16
E4_NP = ml_dtypes.float8_e4m3

B = 512
N_TOK = 32
IN1 = 256
HID = [512, 256, 128]
R = 8
N_CORES = 8

D_AO = 2 * IN1                     # 512
DIMS = [D_AO] + HID                # [512, 512, 256, 128]
WSCALE = 16.0
DR = mybir.MatmulPerfMode.DoubleRow
LR = 0.01                          # leaky-relu slope
BOFF = [0, 4, 6]                   # bias col offset per layer in [128,7]


def _chunks(tp):
    """Processing order: tail chunk first, then full 512s, last full chunk
    split into two 256 halves (halves the drain latency chains)."""
    fulls = []
    s = 0
    while tp - s > 512:
        fulls.append((s, 512))
        s += 512
    tail = (s, tp - s)
    la, lb = fulls[-1]
    return [tail] + fulls[:-1] + [(la, 256), (la + 256, 256)]


def _halves(n):
    if n <= 256:
        return [(0, n)]
    return [(0, 256), (256, n)]


def _build_kernel(tp, pieces):
    """pieces[ci] = list of (group, lo, hi): union spans of b//128 groups
    in chunk ci across all cores (one-hot zeroes wrong-group terms)."""
    nc = bacc.Bacc("TRN2", target_bir_lowering=False, debug=False,
                   num_devices=N_CORES)

    xh_d = nc.dram_tensor("xh", [128, 4, tp], E4, kind="ExternalInput").ap()
    xl_d = nc.dram_tensor("xl", [128, 4, tp], E4, kind="ExternalInput").ap()
    wd = {}
    for nm, shp in (("wh1", [128, 4, 512]), ("wl1", [128, 4, 512]),
                    ("wh2", [128, 4, 256]), ("wl2", [128, 4, 256]),
                    ("wh3", [128, 2, 128]), ("wl3", [128, 2, 128]),
                    ("uiTh", [128, 4, B]), ("uiTl", [128, 4, B]),
                    ("wuh1", [128, 4, 512]), ("wul1", [128, 4, 512])):
        wd[nm] = nc.dram_tensor(nm, shp, E4, kind="ExternalInput").ap()
    wu2_d = nc.dram_tensor("wu2", [128, 4, 256], BF16,
                           kind="ExternalInput").ap()
    wu3_d = nc.dram_tensor("wu3", [128, 2, 128], BF16,
                           kind="ExternalInput").ap()
    bp_d = nc.dram_tensor("bpack", [128, 7], F32, kind="ExternalInput").ap()
    bup_d = nc.dram_tensor("bupack", [128, 7], F32, kind="ExternalInput").ap()
    bids_d = nc.dram_tensor("bids", [128, tp], F16, kind="ExternalInput").ap()
    iota_d = nc.dram_tensor("iota4", [128, 4], F32, kind="ExternalInput").ap()
    pred_d = nc.dram_tensor("pred", [1, tp], F32, kind="ExternalOutput").ap()

    chunks = _chunks(tp)
    NCH = len(chunks)

    with tile.TileContext(nc) as tc:
        with (
            tc.tile_pool(name="wts", bufs=1) as wts,
            tc.tile_pool(name="xin", bufs=1) as xin,
            tc.tile_pool(name="uip", bufs=1) as uip,
            tc.tile_pool(name="actp", bufs=2) as actp,
            tc.tile_pool(name="finp", bufs=2) as finp,
            tc.tile_pool(name="mmps", bufs=8, space="PSUM") as mmps,
        ):
            # ---- tiny PE warm-up (pins pe_busy_start; ~free) ----
            dum = uip.tile([1, 8], BF16, tag="dum")
            nc.vector.memset(dum[:], 0.0)
            psw = mmps.tile([128, 512], F32, tag="mm", name="psw")
            for _ in range(2):
                nc.tensor.matmul(psw[0:1, 0:8], dum[0:1, 0:1], dum[:],
                                 start=True, stop=True)

            # ---- DMAs ----
            # SP queue: UI-phase-critical tensors in half (k-pair) pieces so
            # the first UI matmuls can start ~1.9us in.
            t_uiTh = wts.tile([128, 4, B], E4, tag="uiTh")
            nc.sync.dma_start(t_uiTh[:, 0:2, :], wd["uiTh"][:, 0:2, :])
            t_wul1 = wts.tile([128, 4, 512], E4, tag="wul1")
            nc.sync.dma_start(t_wul1[:, 0:2, :], wd["wul1"][:, 0:2, :])
            nc.sync.dma_start(t_uiTh[:, 2:4, :], wd["uiTh"][:, 2:4, :])
            nc.sync.dma_start(t_wul1[:, 2:4, :], wd["wul1"][:, 2:4, :])
            t_uiTl = wts.tile([128, 4, B], E4, tag="uiTl")
            nc.sync.dma_start(t_uiTl[:], wd["uiTl"])
            t_wu2 = wts.tile([128, 4, 256], BF16, tag="wu2")
            nc.sync.dma_start(t_wu2[:], wu2_d)
            t_wu3 = wts.tile([128, 2, 128], BF16, tag="wu3")
            nc.sync.dma_start(t_wu3[:], wu3_d)
            t_xl = xin.tile([128, 4, tp], E4, tag="xl")
            nc.sync.dma_start(t_xl[:], xl_d)
            t_bids = wts.tile([128, tp], F16, tag="bids")
            nc.sync.dma_start(t_bids[:], bids_d)

            # Act queue: LUT warm-up only (keeps the Act engine free; the
            # auto-inserted table load gets hoisted to the queue head).
            lut = uip.tile([1, 1], F32, tag="lut")
            nc.vector.memset(lut[:], 0.0)
            nc.scalar.activation(lut[:], lut[:],
                                 mybir.ActivationFunctionType.Lrelu,
                                 bias=0.0, scale=1.0, alpha=LR)

            # Pool queue: UI stationary halves, expert weights, x-hi, biases.
            t_wuh1 = wts.tile([128, 4, 512], E4, tag="wuh1")
            nc.gpsimd.dma_start(t_wuh1[:, 0:2, :], wd["wuh1"][:, 0:2, :])
            nc.gpsimd.dma_start(t_wuh1[:, 2:4, :], wd["wuh1"][:, 2:4, :])
            t_bu = wts.tile([128, 7], F32, tag="bu")
            nc.gpsimd.dma_start(t_bu[:], bup_d)
            t_wh1 = wts.tile([128, 4, 512], E4, tag="wh1")
            nc.gpsimd.dma_start(t_wh1[:], wd["wh1"])
            t_xh = xin.tile([128, 4, tp], E4, tag="xh")
            nc.gpsimd.dma_start(t_xh[:], xh_d)
            t_wl1 = wts.tile([128, 4, 512], E4, tag="wl1")
            nc.gpsimd.dma_start(t_wl1[:], wd["wl1"])
            t_b = wts.tile([128, 7], F32, tag="b")
            nc.gpsimd.dma_start(t_b[:], bp_d)
            t_iota = wts.tile([128, 4], F32, tag="iota")
            nc.gpsimd.dma_start(t_iota[:], iota_d)
            t_wh2 = wts.tile([128, 4, 256], E4, tag="wh2")
            nc.gpsimd.dma_start(t_wh2[:], wd["wh2"])
            t_wl2 = wts.tile([128, 4, 256], E4, tag="wl2")
            nc.gpsimd.dma_start(t_wl2[:], wd["wl2"])
            t_wh3 = wts.tile([128, 2, 128], E4, tag="wh3")
            nc.gpsimd.dma_start(t_wh3[:], wd["wh3"])
            t_wl3 = wts.tile([128, 2, 128], E4, tag="wl3")
            nc.gpsimd.dma_start(t_wl3[:], wd["wl3"])

            ones = uip.tile([128, 1], BF16, tag="ones")
            nc.vector.memset(ones[:], 1.0)
            ident = uip.tile([128, 128], F32, tag="ident")
            make_identity(nc, ident[:])

            # ---- emission helpers ----
            def mm3(ps, whT, wlT, mvh, mvl, pair, m, lo, hi):
                """3-term fp8 DoubleRow group piece for one (m, colhalf)."""
                wsl_h = whT[:, 2 * pair:2 * pair + 2, m * 128:(m + 1) * 128]
                wsl_l = wlT[:, 2 * pair:2 * pair + 2, m * 128:(m + 1) * 128]
                xs_h = mvh[:, 2 * pair:2 * pair + 2, lo:hi]
                xs_l = mvl[:, 2 * pair:2 * pair + 2, lo:hi]
                o = ps[:, lo:hi]
                nc.tensor.matmul(o, wsl_h, xs_h, start=(pair == 0),
                                 stop=False, perf_mode=DR)
                nc.tensor.matmul(o, wsl_l, xs_h, start=False, stop=False,
                                 perf_mode=DR)
                nc.tensor.matmul(o, wsl_h, xs_l, start=False,
                                 stop=(pair == 1), perf_mode=DR)

            def mm2(ps, whT, wlT, mv, pairs, m, lo, hi):
                """2-term fp8 DoubleRow group: (wh+wl)*h."""
                o = ps[:, lo:hi]
                for p in range(pairs):
                    wsl_h = whT[:, 2 * p:2 * p + 2, m * 128:(m + 1) * 128]
                    wsl_l = wlT[:, 2 * p:2 * p + 2, m * 128:(m + 1) * 128]
                    xs = mv[:, 2 * p:2 * p + 2, lo:hi]
                    nc.tensor.matmul(o, wsl_h, xs, start=(p == 0),
                                     stop=False, perf_mode=DR)
                    nc.tensor.matmul(o, wsl_l, xs, start=False,
                                     stop=(p == pairs - 1), perf_mode=DR)

            import itertools
            _ctr = itertools.count()

            def act(out, ps, bias_t, bcol, n, scale, dve=False):
                """bias + leaky-relu (+descale); on Act engine or DVE."""
                if not dve:
                    nc.scalar.activation(
                        out, ps, mybir.ActivationFunctionType.Lrelu,
                        bias=bias_t[:, bcol:bcol + 1], scale=scale, alpha=LR)
                else:
                    tmp = actp.tile([128, 512], F32, tag="dvetmp",
                                    name=f"dt{next(_ctr)}")
                    nc.vector.tensor_scalar(
                        out=tmp[:, :n], in0=ps, scalar1=scale,
                        scalar2=bias_t[:, bcol:bcol + 1],
                        op0=mybir.AluOpType.mult, op1=mybir.AluOpType.add)
                    nc.vector.scalar_tensor_tensor(
                        out=out, in0=tmp[:, :n], scalar=LR, in1=tmp[:, :n],
                        op0=mybir.AluOpType.mult, op1=mybir.AluOpType.max)

            S16 = 1.0 / WSCALE

            # ---------------- expert chunk pipeline ----------------
            ps_l1 = [[None] * 4 for _ in range(NCH)]
            ps_l2 = [[None] * 2 for _ in range(NCH)]
            ps_l3 = [None] * NCH
            ps_dot = [None] * NCH
            h1 = [None] * NCH
            h2 = [None] * NCH
            h3 = [None] * NCH
            ohs = [None] * NCH
            prods = [None] * NCH
            pcs = [None] * NCH

            def alloc_chunk(ci):
                for m in range(4):
                    ps_l1[ci][m] = mmps.tile([128, 512], F32, tag="mm",
                                             name=f"ps1c{ci}m{m}")
                for m in range(2):
                    ps_l2[ci][m] = mmps.tile([128, 512], F32, tag="mm",
                                             name=f"ps2c{ci}m{m}")
                ps_l3[ci] = mmps.tile([128, 512], F32, tag="mm",
                                      name=f"ps3c{ci}")
                ps_dot[ci] = mmps.tile([128, 512], F32, tag="mm",
                                       name=f"psdc{ci}")
                h1[ci] = actp.tile([128, 4, 512], E4, tag="h1",
                                   name=f"h1c{ci}")
                h2[ci] = actp.tile([128, 2, 512], E4, tag="h2",
                                   name=f"h2c{ci}")
                h3[ci] = actp.tile([128, 1, 512], BF16, tag="h3",
                                   name=f"h3c{ci}")
                ohs[ci] = finp.tile([128, 4, 512], BF16, tag="oh",
                                    name=f"oh{ci}")
                prods[ci] = finp.tile([128, 512], BF16, tag="prod",
                                      name=f"prod{ci}")
                pcs[ci] = finp.tile([1, 512], F32, tag="pc", name=f"pc{ci}")

            def e_l1_group(ci, m):
                s0, n = chunks[ci]
                for lo, hi in _halves(n):
                    mm3(ps_l1[ci][m], t_wh1, t_wl1,
                        t_xh[:, :, s0:s0 + n], t_xl[:, :, s0:s0 + n],
                        0, m, lo, hi)
                    mm3(ps_l1[ci][m], t_wh1, t_wl1,
                        t_xh[:, :, s0:s0 + n], t_xl[:, :, s0:s0 + n],
                        1, m, lo, hi)

            def e_l1_act(ci, m, dve=False):
                n = chunks[ci][1]
                act(h1[ci][:, m, :n], ps_l1[ci][m][:, :n], t_b,
                    BOFF[0] + m, n, S16, dve=dve)

            def e_l2_pair(ci, m):
                n = chunks[ci][1]
                for lo, hi in _halves(n):
                    mm2(ps_l2[ci][m], t_wh2, t_wl2, h1[ci][:, :, :n],
                        2, m, lo, hi)

            def e_l2_act(ci, m):
                n = chunks[ci][1]
                act(h2[ci][:, m, :n], ps_l2[ci][m][:, :n], t_b,
                    BOFF[1] + m, n, S16)

            def e_l3(ci):
                n = chunks[ci][1]
                for lo, hi in _halves(n):
                    mm2(ps_l3[ci], t_wh3, t_wl3, h2[ci][:, :, :n], 1, 0,
                        lo, hi)

            def e_h3_act(ci):
                # drain chunks put h3 on the (idle) Act engine
                n = chunks[ci][1]
                act(h3[ci][:, 0, :n], ps_l3[ci][:, :n], t_b, BOFF[2], n, S16,
                    dve=(ci < NCH - 2))

            def e_oh(ci):
                s0, n = chunks[ci]
                for j, (g, lo, hi) in enumerate(pieces[ci]["groups"]):
                    nc.vector.tensor_scalar(
                        out=ohs[ci][:, j, lo:hi],
                        in0=t_bids[:, s0 + lo:s0 + hi],
                        scalar1=t_iota[:, g:g + 1], scalar2=None,
                        op0=mybir.AluOpType.is_equal)

            def e_psg(ci):
                mms = pieces[ci]["mms"]
                for j, (slot, lo, hi) in enumerate(mms):
                    g = pieces[ci]["groups"][slot][0]
                    nc.tensor.matmul(
                        ps_dot[ci][:, lo:hi], ui3_tok[:, g, :],
                        ohs[ci][:, slot, lo:hi],
                        start=(j == 0), stop=(j == len(mms) - 1))

            def e_prod(ci):
                n = chunks[ci][1]
                nc.vector.tensor_tensor(
                    out=prods[ci][:, :n], in0=h3[ci][:, 0, :n],
                    in1=ps_dot[ci][:, :n], op=mybir.AluOpType.mult)

            def e_psd(ci):
                n = chunks[ci][1]
                nc.tensor.matmul(ps_dot[ci][0:1, :n], ones[:],
                                 prods[ci][:, :n], start=True, stop=True)

            def e_pc_dma(ci):
                # Pool/GpSimd cannot read PSUM on TRN2 — copy on DVE; the
                # second-to-last chunk drains via the Act engine instead
                # (Lrelu with alpha=1 is an identity copy, same LUT).
                s0, n = chunks[ci]
                if ci == NCH - 2:
                    nc.scalar.activation(pcs[ci][:, :n], ps_dot[ci][0:1, :n],
                                         mybir.ActivationFunctionType.Lrelu,
                                         bias=0.0, scale=1.0, alpha=1.0)
                    nc.scalar.dma_start(pred_d[:, s0:s0 + n], pcs[ci][:, :n])
                else:
                    nc.vector.tensor_copy(pcs[ci][:, :n], ps_dot[ci][0:1, :n])
                    nc.sync.dma_start(pred_d[:, s0:s0 + n], pcs[ci][:, :n])

            for ci in range(NCH):
                alloc_chunk(ci)

            # ---------------- UI phase, interleaved with ch0/ch1 L1 ----
            ps_ui1 = [mmps.tile([128, 512], F32, tag="mm", name=f"psu1m{m}")
                      for m in range(4)]
            ui1 = uip.tile([128, 4, B], BF16, tag="ui1")
            for m in range(4):
                for lo, hi in _halves(B):
                    mm3(ps_ui1[m], t_wuh1, t_wul1, t_uiTh, t_uiTl, 0, m, lo, hi)
                    mm3(ps_ui1[m], t_wuh1, t_wul1, t_uiTh, t_uiTl, 1, m, lo, hi)
                act(ui1[:, m, :], ps_ui1[m][:, :B], t_bu, BOFF[0] + m, B, S16,
                    dve=(m == 3))

            ps_ui2 = [mmps.tile([128, 512], F32, tag="mm", name=f"psu2m{m}")
                      for m in range(2)]
            ui2 = uip.tile([128, 2, B], BF16, tag="ui2")
            for k in range(4):
                for m in range(2):
                    nc.tensor.matmul(
                        ps_ui2[m][:, :B], t_wu2[:, k, m * 128:(m + 1) * 128],
                        ui1[:, k, :], start=(k == 0), stop=(k == 3))
            for m in range(2):
                act(ui2[:, m, :], ps_ui2[m][:, :B], t_bu, BOFF[1] + m, B, 1.0)

            # fillers while ui2/ui3 acts drain: tail chunk L1 + ch1 L1
            for m in range(4):
                e_l1_group(0, m)
                e_l1_act(0, m)
            e_l1_group(1, 0)

            ps_ui3 = mmps.tile([128, 512], F32, tag="mm", name="psu3")
            ui3 = uip.tile([128, 1, B], F32, tag="ui3")
            for k in range(2):
                nc.tensor.matmul(ps_ui3[:, :B], t_wu3[:, k, :], ui2[:, k, :],
                                 start=(k == 0), stop=(k == 1))
            act(ui3[:, 0, :], ps_ui3[:, :B], t_bu, BOFF[2], B, 1.0)

            e_l1_group(1, 1)
            e_l1_act(1, 0)
            e_l1_group(1, 2)
            e_l1_act(1, 1)

            # transpose ui3 -> token-major [128b, 4, 128d]
            tps = mmps.tile([128, 512], F32, tag="mm", name="tps")
            for c in range(4):
                nc.tensor.transpose(tps[:, c * 128:(c + 1) * 128],
                                    ui3[:, 0, c * 128:(c + 1) * 128], ident[:])
            ui3_tok = uip.tile([128, 4, 128], BF16, tag="ui3tok")
            nc.vector.tensor_copy(ui3_tok[:], tps[:])
            for ci in range(NCH):
                e_oh(ci)

            # ch0 (tail) through L2/L3; finish ch1 L1
            e_l2_pair(0, 0)
            e_l2_act(0, 0)
            e_l2_pair(0, 1)
            e_l2_act(0, 1)
            e_l1_group(1, 3)
            e_l1_act(1, 2)
            e_l1_act(1, 3)
            e_l3(0)
            e_h3_act(0)

            # ---- steady periods + drain ----
            # period p: L1(p+1) | L2+L3(p) | psg/prod/psd/pc(p-1)
            for p in range(1, NCH + 1):
                cl1 = p + 1 if p + 1 < NCH else None
                cl2 = p if p < NCH else None
                cfin = p - 1
                if cl1 is not None:
                    e_l1_group(cl1, 0)
                if cl2 is not None:
                    e_l2_pair(cl2, 0)
                    e_l2_act(cl2, 0)
                e_psg(cfin)
                if cl1 is not None:
                    e_l1_group(cl1, 1)
                    e_l1_act(cl1, 0)
                if cl2 is not None:
                    e_l2_pair(cl2, 1)
                    e_l2_act(cl2, 1)
                e_prod(cfin)
                if cl1 is not None:
                    e_l1_group(cl1, 2)
                    e_l1_act(cl1, 1)
                e_psd(cfin)
                e_pc_dma(cfin)
                if cl1 is not None:
                    e_l1_group(cl1, 3)
                    e_l1_act(cl1, 2)
                    e_l1_act(cl1, 3)
                if cl2 is not None:
                    e_l3(cl2)
                    e_h3_act(cl2)

    nc.compile()
    return nc


def _prepare(u_emb, i_emb, a_emb, o_emb, s):
    """Host-side sharding + fp8 hi/lo splits + layouts."""
    s_flat = np.asarray(s).reshape(-1).astype(np.int64)
    n_tokens = s_flat.shape[0]
    X = np.concatenate(
        [np.asarray(a_emb, dtype=np.float32).reshape(n_tokens, IN1),
         np.asarray(o_emb, dtype=np.float32).reshape(n_tokens, IN1)],
        axis=1)
    uiT = np.ascontiguousarray(
        np.concatenate([np.asarray(u_emb, dtype=np.float32),
                        np.asarray(i_emb, dtype=np.float32)], axis=1).T)

    idx = [np.flatnonzero(s_flat == e) for e in range(R)]
    tp = max(520, -(-max(max(len(ix) for ix in idx), 1) // 8) * 8)
    chunks = _chunks(tp)

    iota4 = np.ascontiguousarray(
        (np.arange(128, dtype=np.float32)[:, None]
         + 128.0 * np.arange(4, dtype=np.float32)[None, :]))

    def lay(a, kc):
        return np.ascontiguousarray(a.reshape(kc, 128, -1).transpose(1, 0, 2))

    uiTh = uiT.astype(E4_NP)
    uiTl = (uiT - uiTh.astype(np.float32)).astype(E4_NP)

    in_maps = []
    runs_all = [[] for _ in chunks]
    for e in range(R):
        order = np.argsort(idx[e] // N_TOK, kind="stable")
        idx[e] = idx[e][order]
        ix = idx[e]
        pad = np.full(tp, n_tokens - 1, dtype=np.int64)
        pad[:len(ix)] = ix
        xT = np.ascontiguousarray(X[pad].T)          # [512, tp]
        xh = xT.astype(E4_NP)
        xl = (xT - xh.astype(np.float32)).astype(E4_NP)
        b_of_tok = pad // N_TOK
        gid = b_of_tok // 128
        bids = np.ascontiguousarray(np.broadcast_to(
            b_of_tok.astype(np.float16)[None, :], (128, tp)))
        for ci, (s0, n) in enumerate(chunks):
            g = gid[s0:s0 + n]
            start = 0
            for j in range(1, n + 1):
                if j == n or g[j] != g[start]:
                    runs_all[ci].append((int(g[start]), start, j))
                    start = j
        in_maps.append({"xh": lay(xh, 4), "xl": lay(xl, 4), "bids": bids,
                        "iota4": iota4, "uiTh": lay(uiTh, 4),
                        "uiTl": lay(uiTl, 4)})

    # union spans per (chunk, group) across cores; psg matmuls split at
    # overlap boundaries (each mm range uniformly fresh or accumulating)
    pieces = []
    for ci in range(len(chunks)):
        spans = {}
        for g, lo, hi in runs_all[ci]:
            if g in spans:
                spans[g] = (min(spans[g][0], lo), max(spans[g][1], hi))
            else:
                spans[g] = (lo, hi)
        groups = sorted((g, lo, hi) for g, (lo, hi) in spans.items())
        bounds = sorted({b for _, lo, hi in groups for b in (lo, hi)})
        mms = []
        for a, b in zip(bounds[:-1], bounds[1:]):
            for slot, (g, lo, hi) in enumerate(groups):
                if lo <= a and b <= hi:
                    mms.append((slot, a, b))
        pieces.append({"groups": groups, "mms": mms})

    return in_maps, idx, tp, pieces


def _split16(w):
    ws = np.asarray(w, dtype=np.float32) * WSCALE
    wh = ws.astype(E4_NP)
    wl = (ws - wh.astype(np.float32)).astype(E4_NP)
    return wh, wl


def _lay(a, kc):
    return np.ascontiguousarray(
        np.asarray(a).reshape(kc, 128, -1).transpose(1, 0, 2))


def _pack_bias(bs):
    cols = []
    for b in bs:
        cols.append(np.asarray(b, dtype=np.float32).reshape(-1, 128).T)
    return np.ascontiguousarray(np.concatenate(cols, axis=1))


def _add_weights(m, e, W1, b1, W2, b2, W3, b3, Wu1, bu1, Wu2, bu2, Wu3, bu3,
                 bupack, wu_cache):
    for nm, w, kc in (("w1", W1[e], 4), ("w2", W2[e], 4), ("w3", W3[e], 2)):
        wh, wl = _split16(w)
        m[nm.replace("w", "wh")] = _lay(wh, kc)
        m[nm.replace("w", "wl")] = _lay(wl, kc)
    if not wu_cache:
        wh, wl = _split16(Wu1)
        wu_cache["wuh1"] = _lay(wh, 4)
        wu_cache["wul1"] = _lay(wl, 4)
        wu_cache["wu2"] = _lay(
            np.asarray(Wu2, dtype=np.float32).astype(BF16_NP), 4)
        wu_cache["wu3"] = _lay(
            np.asarray(Wu3, dtype=np.float32).astype(BF16_NP), 2)
    m.update(wu_cache)
    m["bpack"] = _pack_bias([b1[e], b2[e], b3[e]])
    m["bupack"] = bupack


def kernel(u_emb, i_emb, a_emb, o_emb, s,
           W1, b1, W2, b2, W3, b3,
           Wu1, bu1, Wu2, bu2, Wu3, bu3):
    in_maps, idx, tp, pieces = _prepare(u_emb, i_emb, a_emb, o_emb, s)
    bupack = _pack_bias([bu1, bu2, bu3])
    wu_cache = {}
    for e in range(R):
        _add_weights(in_maps[e], e, W1, b1, W2, b2, W3, b3,
                     Wu1, bu1, Wu2, bu2, Wu3, bu3, bupack, wu_cache)

    nc = _build_kernel(tp, pieces)
    res = run_bass_kernel_spmd(nc, in_maps, core_ids=list(range(N_CORES)))

    s_arr = np.asarray(s)
    out = np.zeros(s_arr.size, dtype=np.float32)
    for e in range(R):
        pred = res.results[e]["pred"].reshape(-1)
        out[idx[e]] = pred[:len(idx[e])]
    return out.reshape(s_arr.shape)
